# revision 1
# baseline (speedup 1.0000x reference)
"""BesselKAN layer kernel for Trainium2 (8 NeuronCores, data-parallel over batch).

reference math:
    t = tanh(x)                                   # [B, I]
    b0 = 1; b1 = t + 1; b2 = 3t*b1 + b0; b3 = 5t*b2 + b1
    y[b,o] = sum_{i,d} b_d[b,i] * W[i,o,d]        # W = bessel_coeffs [I, O, 4]

Monomial rewrite (exact algebra):
    b0 = 1
    b1 = 1 + t
    b2 = 1 + 3t + 3t^2
    b3 = 1 + 6t + 15t^2 + 15t^3
    y  = 1 @ (W0+W1+W2+W3)            -> bias row, rank-1: bias_o = colsum
       + t        @ (W1 + 3W2 + 6W3)  =: u1 @ C1
       + (3 t^2)  @ (W2 + 5W3)        =: u2 @ C2
       + (15 t^3) @ W3                =: u3 @ C3

So per core (1024 batch rows): 3 bf16 matmuls of [1024,1024]x[1024,1024]
plus a broadcast bias add.  The bias is computed with an all-ones f32r
stationary matmul over the raw fp32 W chunks (ones^T @ W broadcasts the
column-sum into every PSUM partition).
"""

import os
import sys

import numpy as np

if "/opt/trn_rl_repo" not in sys.path:
    sys.path.insert(0, "/opt/trn_rl_repo")

from contextlib import ExitStack

import concourse.bass as bass
import concourse.tile as tile
from concourse import bacc, mybir
from concourse._compat import with_exitstack
from concourse.masks import make_identity

P = 128
N_CORES = 8
B_FULL = 8192
I_DIM = 1024
O_DIM = 1024
NDEG = 4  # D+1

FP32 = mybir.dt.float32
F32R = mybir.dt.float32r
BF16 = mybir.dt.float16


DEFAULT_CFG = dict(
    nsplit=2,       # x DMA/tanh split per row-block
    ga=2,           # leading bi-group riding the W stream
    d2=0, d3=0,     # term pipeline delays for u2/u3 (u's are ready early)
    w_pre=True,     # issue first W chunk right after x0
    n_x_early=4,    # row-blocks emitted before the W loop
    u3_k1_double=True,  # emit two u3 ops at ki=1
    j_ramp=2,       # u1 rows issued at chunk 0 (rest catch up at chunk 1)
    xbufs=3, wbufs=6, obufs=3, pobufs=4, tbufs=2, ptbufs=3,
)


@with_exitstack
def _bessel_body(ctx: ExitStack, tc: "tile.TileContext", y_d, x_d, w_d,
                 b_loc, i_dim, o_dim, cfg=None):
    cfg = {**DEFAULT_CFG, **(cfg or {})}
    nc = tc.nc
    BI = b_loc // P           # batch tiles
    KI = i_dim // P           # contraction tiles
    OW = min(512, o_dim)      # matmul moving free size (one PSUM bank fp32)
    OH = o_dim // OW          # output column tiles

    singles = ctx.enter_context(tc.tile_pool(name="singles", bufs=1))
    xpool = ctx.enter_context(tc.tile_pool(name="xpool", bufs=cfg["xbufs"]))
    tpool = ctx.enter_context(tc.tile_pool(name="tpool", bufs=cfg["tbufs"]))
    wpool = ctx.enter_context(tc.tile_pool(name="wpool", bufs=cfg["wbufs"]))
    opool = ctx.enter_context(tc.tile_pool(name="opool", bufs=cfg["obufs"]))
    psum_t = ctx.enter_context(
        tc.tile_pool(name="psum_t", bufs=cfg["ptbufs"], space="PSUM"))
    psum_b = ctx.enter_context(tc.tile_pool(name="psum_b", bufs=1, space="PSUM"))
    psum_o = ctx.enter_context(
        tc.tile_pool(name="psum_o", bufs=cfg["pobufs"], space="PSUM"))

    identity16 = singles.tile([P, P], BF16, name="identity16")
    make_identity(nc, identity16)
    # All-λ stationary matrices: (λ·ones)^T @ M broadcasts λ·colsum(M) into
    # every PSUM partition.  bias = colsum(W0) + colsum(C1 - 2*C2 + 5*C3)
    # because C1 - 2*C2 + 5*C3 == W1 + W2 + W3.
    ones_bf = singles.tile([P, P], BF16, name="ones_bf")
    neg2_bf = singles.tile([P, P], BF16, name="neg2_bf")
    five_bf = singles.tile([P, P], BF16, name="five_bf")
    nc.vector.memset(ones_bf[:], 1.0)
    nc.vector.memset(neg2_bf[:], -2.0)
    nc.vector.memset(five_bf[:], 5.0)

    # Persistent basis (u, [i_part, ki, b]) and combined weights (C, [i_part, ki, o]).
    u1 = singles.tile([P, KI, b_loc], BF16, name="u1")
    u2 = singles.tile([P, KI, b_loc], BF16, name="u2")
    u3 = singles.tile([P, KI, b_loc], BF16, name="u3")

    # ---- phase X: tanh (bf16) -> PE transpose -> u1; u2 = 3t^2 (GpSimd).
    # u3 = 5t*u2 is emitted later inside the W stream (DVE) so it neither
    # blocks C-prep ordering nor gates the early A-group matmuls.
    def emit_x_phase(bi):
        bsl = slice(bi * P, (bi + 1) * P)
        x_t = xpool.tile([P, i_dim], FP32, tag="x_t", name=f"x_t{bi}")
        tf = tpool.tile([P, i_dim], BF16, tag="tf", name=f"tf{bi}")
        nsplit = cfg["nsplit"] if bi == 0 else 1
        kstep = KI // nsplit
        for s in range(nsplit):
            ssl = slice(s * kstep * P, (s + 1) * kstep * P)
            nc.sync.dma_start(out=x_t[:, ssl],
                              in_=x_d[bi * P:(bi + 1) * P, ssl])
            nc.scalar.activation(out=tf[:, ssl], in_=x_t[:, ssl],
                                 func=mybir.ActivationFunctionType.Tanh)
            for ki in range(s * kstep, (s + 1) * kstep):
                ps = psum_t.tile([P, P], BF16, tag="ps_t",
                                 name=f"ps_t{bi}_{ki}")
                nc.tensor.transpose(ps[:], tf[:, ki * P:(ki + 1) * P],
                                    identity16[:])
                nc.scalar.copy(u1[:, ki, bsl], ps[:])
        u1s = u1[:, :, bsl]
        nc.vector.scalar_tensor_tensor(
            out=u2[:, :, bsl], in0=u1s, scalar=3.0, in1=u1s,
            op0=mybir.AluOpType.mult, op1=mybir.AluOpType.mult,
        )
        nc.vector.scalar_tensor_tensor(
            out=u3[:, :, bsl], in0=u1s, scalar=5.0, in1=u2[:, :, bsl],
            op0=mybir.AluOpType.mult, op1=mybir.AluOpType.mult,
        )

    def emit_u3(bi):
        bsl = slice(bi * P, (bi + 1) * P)
        nc.vector.scalar_tensor_tensor(
            out=u3[:, :, bsl], in0=u1[:, :, bsl], scalar=5.0,
            in1=u2[:, :, bsl],
            op0=mybir.AluOpType.mult, op1=mybir.AluOpType.mult,
        )

    def issue_w(oh, ki):
        w_t = wpool.tile([P, OW, NDEG], FP32, tag="w_t", name=f"w_t{oh}_{ki}")
        nc.sync.dma_start(
            out=w_t[:],
            in_=w_d[ki * P:(ki + 1) * P, oh * OW:(oh + 1) * OW, :])
        return w_t

    n_x_emitted = min(cfg["n_x_early"], BI)
    w_pre = None
    for bi in range(n_x_emitted):
        emit_x_phase(bi)
        if bi == 0 and cfg["w_pre"]:
            # first W chunk rides right behind x0 so C[0] is ready by the
            # time the transposes drain
            w_pre = issue_w(0, 0)
    u3_pending = []

    # ---- phases W+MAIN, one o-column half at a time so the second half's
    # W stream overlaps the first half's matmuls.  Separate C/bias tensors
    # per half avoid false WAR deps in Tile's access tracking.
    GA = min(cfg["ga"], BI)  # leading bi-group interleaved with W stream
    for oh in range(OH):
        osl = slice(oh * OW, (oh + 1) * OW)
        c1 = singles.tile([P, KI, OW], BF16, name=f"c1_{oh}")
        c2 = singles.tile([P, KI, OW], BF16, name=f"c2_{oh}")
        c3 = singles.tile([P, KI, OW], BF16, name=f"c3_{oh}")
        bias_ps = psum_b.tile([P, OW], FP32, tag="bias_ps",
                              name=f"bias_ps{oh}")
        bias = singles.tile([P, OW], FP32, name=f"bias{oh}")
        terms = ((u1, c1), (u2, c2), (u3, c3))
        pos_a = [psum_o.tile([P, OW], FP32, tag="po", name=f"po_a{oh}_{j}")
                 for j in range(GA)]
        D = (0, cfg["d2"], cfg["d3"])  # per-term chunk delays
        a_started = [False] * GA

        def a_mms(ti, kk, js):
            u, cc = terms[ti]
            for j in js:
                nc.tensor.matmul(
                    pos_a[j][:],
                    u[:, kk, j * P:(j + 1) * P],
                    cc[:, kk, :],
                    start=not a_started[j],
                    stop=(ti == len(terms) - 1 and kk == KI - 1),
                )
                a_started[j] = True

        def a_step(c):
            # software-pipelined A-group accumulation step for chunk index c.
            # At c=0 (first half only) just j0/j1 — later rows' tanh results
            # aren't in yet and an in-order PE stall would block everything;
            # the skipped pairs catch up at c=1.
            for ti, (u, cc) in enumerate(terms):
                kk = c - D[ti]
                if not 0 <= kk < KI:
                    continue
                jr = cfg["j_ramp"]
                if ti == 0 and c == 0 and oh == 0 and GA > jr:
                    a_mms(ti, kk, range(jr))
                else:
                    a_mms(ti, kk, range(GA))
            if c == 1 and oh == 0 and GA > cfg["j_ramp"]:
                a_mms(0, 0, range(cfg["j_ramp"], GA))

        for ki in range(KI):
            if oh == 0 and ki >= 1 and u3_pending:
                emit_u3(u3_pending.pop(0))
                if ki == 1 and cfg["u3_k1_double"] and u3_pending:
                    emit_u3(u3_pending.pop(0))
            w_t = w_pre if (oh == 0 and ki == 0 and w_pre is not None) \
                else issue_w(oh, ki)
            w1 = w_t[:, :, 1]
            w2 = w_t[:, :, 2]
            w3 = w_t[:, :, 3]
            tmp = wpool.tile([P, OW], FP32, tag="tmpc")
            # c1 = w1 + 3*w2 + 6*w3 ; c2 = w2 + 5*w3 ; c3 = w3
            nc.vector.scalar_tensor_tensor(
                out=tmp[:], in0=w2, scalar=3.0, in1=w1,
                op0=mybir.AluOpType.mult, op1=mybir.AluOpType.add,
            )
            nc.vector.scalar_tensor_tensor(
                out=c1[:, ki, :], in0=w3, scalar=6.0, in1=tmp[:],
                op0=mybir.AluOpType.mult, op1=mybir.AluOpType.add,
            )
            nc.vector.scalar_tensor_tensor(
                out=c2[:, ki, :], in0=w3, scalar=5.0, in1=w2,
                op0=mybir.AluOpType.mult, op1=mybir.AluOpType.add,
            )
            nc.scalar.copy(c3[:, ki, :], w3)
            w0_bf = wpool.tile([P, OW], BF16, tag="w0_bf")
            nc.scalar.copy(w0_bf[:], w_t[:, :, 0])
            # leading bi-group rides the W stream (terms pipelined so late
            # u2/u3 availability never stalls the PE).
            a_step(ki)
            # bias accumulation over ki: colsum(W0) + colsum(C1 - 2*C2 + 5*C3)
            movers = (w0_bf[:], c1[:, ki, :], c2[:, ki, :], c3[:, ki, :])
            lhs = (ones_bf, ones_bf, neg2_bf, five_bf)
            for t in range(4):
                nc.tensor.matmul(
                    bias_ps[:],
                    lhs[t][:],
                    movers[t],
                    start=(ki == 0 and t == 0),
                    stop=(ki == KI - 1 and t == 3),
                )
        while oh == 0 and n_x_emitted < BI:
            emit_x_phase(n_x_emitted)
            n_x_emitted += 1
        while oh == 0 and u3_pending:
            emit_u3(u3_pending.pop(0))
        for c in range(KI, KI + D[-1]):
            a_step(c)
        nc.vector.tensor_copy(bias[:], bias_ps[:])
        for j in range(GA):
            yo = opool.tile([P, OW], FP32, tag="yo")
            nc.vector.tensor_add(yo[:], pos_a[j][:], bias[:])
            nc.sync.dma_start(out=y_d[j * P:(j + 1) * P, osl], in_=yo[:])

        # trailing bi-groups: all C for this half is resident, full speed.
        for bi in range(GA, BI):
            bsl = slice(bi * P, (bi + 1) * P)
            po = psum_o.tile([P, OW], FP32, tag="po")
            for ki in range(KI):
                for ti, (u, c) in enumerate(terms):
                    nc.tensor.matmul(
                        po[:],
                        u[:, ki, bsl],
                        c[:, ki, :],
                        start=(ki == 0 and ti == 0),
                        stop=(ki == KI - 1 and ti == len(terms) - 1),
                    )
            yo = opool.tile([P, OW], FP32, tag="yo")
            nc.vector.tensor_add(yo[:], po[:], bias[:])
            nc.sync.dma_start(out=y_d[bi * P:(bi + 1) * P, osl], in_=yo[:])


def build_nc(b_loc=B_FULL // N_CORES, i_dim=I_DIM, o_dim=O_DIM,
             n_cores=N_CORES, cfg=None):
    nc = bacc.Bacc("TRN2", target_bir_lowering=False, debug=False,
                   num_devices=n_cores)
    x_d = nc.dram_tensor("x", [b_loc, i_dim], FP32, kind="ExternalInput").ap()
    w_d = nc.dram_tensor("w", [i_dim, o_dim, NDEG], FP32,
                         kind="ExternalInput").ap()
    y_d = nc.dram_tensor("y", [b_loc, o_dim], FP32, kind="ExternalOutput").ap()
    with tile.TileContext(nc) as tc:
        _bessel_body(tc, y_d, x_d, w_d, b_loc, i_dim, o_dim, cfg=cfg)
    nc.compile()
    return nc


_NC_CACHE = {}


def _get_nc():
    key = "full"
    if key not in _NC_CACHE:
        _NC_CACHE[key] = build_nc()
    return _NC_CACHE[key]


def run_spmd(x, bessel_coeffs, trace=False, **kwargs):
    """Run the SPMD kernel on 8 cores; returns (y_full, BassKernelResults)."""
    from concourse.bass_utils import run_bass_kernel_spmd

    nc = _get_nc()
    x = np.ascontiguousarray(np.asarray(x, dtype=np.float32))
    w = np.ascontiguousarray(np.asarray(bessel_coeffs, dtype=np.float32))
    b_loc = x.shape[0] // N_CORES
    in_maps = [
        {"x": x[c * b_loc:(c + 1) * b_loc], "w": w} for c in range(N_CORES)
    ]
    res = run_bass_kernel_spmd(nc, in_maps, core_ids=list(range(N_CORES)),
                               trace=trace, **kwargs)
    y = np.concatenate([r["y"] for r in res.results], axis=0)
    return y, res


def kernel(x, bessel_coeffs):
    y, _ = run_spmd(x, bessel_coeffs)
    return y.astype(np.float32)


def _ref_np(x, w):
    t = np.tanh(np.asarray(x, dtype=np.float64))
    w = np.asarray(w, dtype=np.float64)
    basis = [np.ones_like(t), t + 1.0]
    for i in range(2, NDEG):
        basis.append((2 * i - 1) * t * basis[i - 1] + basis[i - 2])
    bz = np.stack(basis, axis=-1)
    return np.einsum("bid,iod->bo", bz, w)


def _selftest_sim(b_loc=256, i_dim=256, o_dim=1024):
    """CoreSim check on a small config exercising all loop paths."""
    from concourse.bass_interp import CoreSim

    nc = build_nc(b_loc=b_loc, i_dim=i_dim, o_dim=o_dim, n_cores=1)
    rng = np.random.default_rng(0)
    x = rng.standard_normal((b_loc, i_dim)).astype(np.float32)
    w = (rng.standard_normal((i_dim, o_dim, NDEG)) / (i_dim * NDEG)).astype(
        np.float32)
    sim = CoreSim(nc)
    sim.tensor("x")[:] = x
    sim.tensor("w")[:] = w
    sim.simulate()
    y = np.array(sim.tensor("y"))
    ref = _ref_np(x, w)
    scale = np.abs(ref).max()
    err = np.abs(y - ref).max() / scale
    print(f"sim scale={scale:.4g} max_abs_rel_err={err:.4g}")
    assert err < 2e-2, err
    print("SIM OK")


if __name__ == "__main__":
    if "--sim" in sys.argv:
        _selftest_sim()



# revision 48
# speedup vs baseline: 1.7048x; 1.7048x over previous
"""BesselKAN layer kernel for Trainium2 (8 NeuronCores, data-parallel batch).

reference math:
    t = tanh(x)                                   # [B, I]
    b0 = 1; b1 = 1+t; b2 = 1+3t+3t^2; b3 = 1+6t+15t^2+15t^3
    y[b,o] = sum_{i,d} b_d[b,i] * W[i,o,d]        # W = bessel_coeffs [I, O, 4]

Monomial rewrite (exact algebra):
    y = bias + t @ C1 + (3 t^2) @ C2 + (15 t^3) @ C3
    C1 = W1 + 3 W2 + 6 W3 ; C2 = W2 + 5 W3 ; C3 = W3
    bias_o = colsum(W0 + W1 + W2 + W3)

Device strategy (per core, 1024 batch rows):
  - All contraction matmuls run as fp8e4m3 DoubleRow (2x PE rate).  The
    t^2/t^3 terms carry most of the signal, so their operands are split
    hi/lo: u = q(u) + r, C = q(C) + r(C), and three DR passes per term
    (q@q, r@q, q@r) recover ~bf16 accuracy while costing 1.5x one fp8
    pass instead of 2x (bf16).
  - Weight-side planes are host-prepared (layout permute + monomial fold
    + 2^13 pre-scale + fp8/bf16 casts): c1q, c2q/c2r, c3q/c3r, ssq/ssr.
    Everything is scaled by 8192 so fp8 residual planes stay in normal
    range; the final PSUM->SBUF copy descales by 1/8192 (exact).
  - x is host-transposed so tanh lands directly in [i_part, b] layout
    (no PE transposes).  u-side tensors: t (tanh, ACT), 3t^2 (ACT Square
    with scale sqrt3), 15t^3 (DVE stt), fp8 copies (DVE tensor_copy, 2x
    mode), residuals (GpSimd stt - otherwise idle engine).
  - bias: colsum via DR matmuls with an all-ones fp8 stationary, then a
    row-select (e_row) bf16 matmul broadcasts it into each PSUM tile as
    the accumulation group's last matmul.
  - y leaves PSUM via ACT Copy (scale 1/8192) as bf16; host upcasts.
"""

import sys
from contextlib import ExitStack

import numpy as np

if "/opt/trn_rl_repo" not in sys.path:
    sys.path.insert(0, "/opt/trn_rl_repo")

import ml_dtypes

import concourse.bass as bass
import concourse.tile as tile
from concourse import bacc, mybir
from concourse._compat import with_exitstack

P = 128
N_CORES = 8
B_FULL = 8192
I_DIM = 1024
O_DIM = 1024
NDEG = 4

FP32 = mybir.dt.float32
BF16 = mybir.dt.bfloat16
FP8 = mybir.dt.float8e4

SQRT3 = float(np.sqrt(3.0))
WSCALE = 8192.0  # weight-plane pre-scale (2^13, exact)

BF16_NP = ml_dtypes.bfloat16
FP8_NP = ml_dtypes.float8_e4m3

MULT = mybir.AluOpType.mult
ADD = mybir.AluOpType.add
SUB = mybir.AluOpType.subtract

DEFAULT_CFG = dict(
    xbufs=4,
    wbufs=4,
    yobufs=4,
    pbufs=7,
    wave_a=6,
    nsplit=2,
    nsplit_kis=2,
    resid_engine="gpsimd",  # or "vector"
    yo_engine="vector",  # vector | scalar2 (gpsimd cannot read PSUM)
    sched="q0 q1 r0 q2 r1 q3 r2 r3",
    w_chunk=True,
    colsum_at=4,
    companions=0,
    splits=0,
    split_from=3,
    w_order=("c1q0 c2q0 c3q0 c2r0 c3r0 c1q0 c2q0 c3q0 c2r0 c3r0 "
             "ssq0 ssr0 c1q1 c2q1 c3q1 c2r1 c3r1 ssq1 ssr1"),
)


@with_exitstack
def _bessel_body(ctx: ExitStack, tc: "tile.TileContext", y_d, xt_d, wplanes_d,
                 b_loc, i_dim, o_dim, cfg=None):
    """wplanes_d: dict name -> dram AP, each [P, KI, o_dim]:
    c1q(fp8) c2q c2r c3q c3r (fp8) ssq ssr (fp8)."""
    cfg = {**DEFAULT_CFG, **(cfg or {})}
    nc = tc.nc
    KI = i_dim // P
    KP = KI // 2
    NJ = b_loc // P
    OW = min(512, o_dim)
    OH = o_dim // OW
    resid = nc.gpsimd if cfg["resid_engine"] == "gpsimd" else nc.vector

    singles = ctx.enter_context(tc.tile_pool(name="singles", bufs=1))
    xpool = ctx.enter_context(tc.tile_pool(name="xpool", bufs=cfg["xbufs"]))
    wpool = ctx.enter_context(tc.tile_pool(name="wpool", bufs=cfg["wbufs"]))
    yopool = ctx.enter_context(tc.tile_pool(name="yopool", bufs=cfg["yobufs"]))
    psum_o = ctx.enter_context(
        tc.tile_pool(name="psum_o", bufs=cfg["pbufs"], space="PSUM"))
    psum_b = ctx.enter_context(
        tc.tile_pool(name="psum_b", bufs=1, space="PSUM"))

    # constants
    ones_dr = singles.tile([P, 2, P], FP8, name="ones_dr")
    nc.vector.memset(ones_dr[:], 1.0)
    e_row = singles.tile([P, P], BF16, name="e_row")
    nc.vector.memset(e_row[:], 0.0)
    nc.vector.memset(e_row[0:1, :], 1.0)

    # u-side persistent tensors, [i_part, ki, b]
    u1b = singles.tile([P, KI, b_loc], BF16, name="u1b")
    u1q = singles.tile([P, KI, b_loc], FP8, name="u1q")
    u2b = singles.tile([P, KI, b_loc], BF16, name="u2b")
    u2q = singles.tile([P, KI, b_loc], FP8, name="u2q")
    u2r = singles.tile([P, KI, b_loc], FP8, name="u2r")
    u3b = singles.tile([P, KI, b_loc], BF16, name="u3b")
    u3q = singles.tile([P, KI, b_loc], FP8, name="u3q")
    u3r = singles.tile([P, KI, b_loc], FP8, name="u3r")

    # weight-side persistent fp8 tiles per output half
    W_NAMES = ("c1q", "c2q", "c2r", "c3q", "c3r")
    wsb = {(n, oh): singles.tile([P, KI, OW], FP8, name=f"{n}_{oh}")
           for n in W_NAMES for oh in range(OH)}
    sssb = {(n, oh): singles.tile([P, KI, OW], FP8, name=f"{n}_{oh}")
            for n in ("ssq", "ssr") for oh in range(OH)}
    bias_sb = [singles.tile([P, OW], BF16, name=f"bias_sb{oh}")
               for oh in range(OH)]
    bias_sc = [singles.tile([P, OW], BF16, name=f"bias_sc{oh}")
               for oh in range(OH)]

    def emit_u_pair(kp, nsplit=1, wtake=()):
        # Emit both kis of a contraction pair with ops grouped by matmul-pass
        # consumption priority (u1q -> u2q -> u3q -> residuals) so each
        # engine's in-order queue produces pair-complete tensors asap.
        # nsplit>1 additionally halves the b-range per op for shorter chain
        # latency at kernel startup.
        kis = [2 * kp, 2 * kp + 1][:max(1, KI - 2 * kp)]
        xts = {}
        for ki in kis:
            xts[ki] = xpool.tile([P, b_loc], BF16, tag="x_t", name=f"x_t{ki}")
        step = b_loc // nsplit
        for s in range(nsplit):
            for _ in range(wtake[s] if s < len(wtake) else 0):
                if wq:
                    emit_wdma(*wq.pop(0))
            bsl = slice(s * step, (s + 1) * step)

            def sl(ki):
                return (slice(None), ki, bsl)

            for ki in kis:
                nc.sync.dma_start(out=xts[ki][:, bsl], in_=xt_d[:, ki, bsl])
                nc.scalar.activation(out=u1b[sl(ki)], in_=xts[ki][:, bsl],
                                     func=mybir.ActivationFunctionType.Tanh)
            for ki in kis:
                nc.vector.tensor_copy(out=u1q[sl(ki)], in_=u1b[sl(ki)])
                nc.scalar.activation(out=u2b[sl(ki)], in_=u1b[sl(ki)],
                                     func=mybir.ActivationFunctionType.Square,
                                     scale=SQRT3)
            for ki in kis:
                nc.gpsimd.tensor_copy(out=u2q[sl(ki)], in_=u2b[sl(ki)])
            for ki in kis:
                nc.vector.scalar_tensor_tensor(
                    out=u3b[sl(ki)], in0=u1b[sl(ki)], scalar=5.0,
                    in1=u2b[sl(ki)], op0=MULT, op1=MULT)
                nc.vector.tensor_copy(out=u3q[sl(ki)], in_=u3b[sl(ki)])
            for ki in kis:
                nc.vector.tensor_tensor(out=u2r[sl(ki)], in0=u2b[sl(ki)],
                                        in1=u2q[sl(ki)], op=SUB)
            for ki in kis:
                nc.gpsimd.tensor_tensor(out=u3r[sl(ki)], in0=u3b[sl(ki)],
                                        in1=u3q[sl(ki)], op=SUB)

    def emit_wdma(name, oh, kis=None):
        dst = sssb[(name, oh)] if name in ("ssq", "ssr") else wsb[(name, oh)]
        kis = kis or (0, KI)
        nc.sync.dma_start(
            out=dst[:, kis[0]:kis[1], :],
            in_=wplanes_d[name][:, kis[0]:kis[1], oh * OW:(oh + 1) * OW])

    # ---- emission: u-prep interleaved with W DMAs (program order ~ priority).
    # q-planes stream first (they gate the early matmul passes), residual
    # planes next, ss planes last (bias is only needed at group close).
    # The first c3q/c2q chunks are split so the first passes' operands land
    # with the first x tiles.
    kc = min(2, KI)
    W_CHUNKED = {"c1q", "c2q", "c3q", "c2r", "c3r"}

    def worder():
        # cfg string: space-separated "<plane><oh>" tokens; chunked planes
        # expand to (0,kc) + (kc,KI) at their first/second occurrence
        seen = set()
        out = []
        for tok in cfg["w_order"].split():
            n, oh = tok[:3], int(tok[3])
            if oh >= OH:
                continue
            if n in W_CHUNKED and cfg["w_chunk"] and oh == 0:
                if (n, oh) not in seen:
                    out.append((n, oh, (0, kc)))
                    seen.add((n, oh))
                else:
                    out.append((n, oh, (kc, KI)))
            elif (n, oh) not in seen:
                out.append((n, oh, None))
                seen.add((n, oh))
        emitted = {}
        for n, oh, k in out:
            lo, hi = k if k else (0, KI)
            emitted[(n, oh)] = max(emitted.get((n, oh), 0), hi)
        need = [(n, oh) for n in
                ("c1q", "c2q", "c2r", "c3q", "c3r", "ssq", "ssr")
                for oh in range(OH)]
        for n, oh in need:
            hi = emitted.get((n, oh), 0)
            if hi < KI:
                out.append((n, oh, (hi, KI)))
        return [e for e in out if e[2] is None or e[2][0] < e[2][1]]

    wq = worder()
    for kp in range(KP):
        if kp == 0:
            emit_u_pair(kp, nsplit=cfg["nsplit"], wtake=(0, 2))
            take = 2
        else:
            emit_u_pair(kp)
            take = 2
        for _ in range(take):
            if wq:
                emit_wdma(*wq.pop(0))
    while wq:
        emit_wdma(*wq.pop(0))

    # (u fp8 tensor, weight plane name) passes per accumulation group,
    # ordered by when the operands become available (q before residual)
    PASSES = (
        ("t1qq", u1q, "c1q"), ("t2qq", u2q, "c2q"), ("t3qq", u3q, "c3q"),
        ("t2qr", u2q, "c2r"), ("t3qr", u3q, "c3r"),
        ("t2rq", u2r, "c2q"), ("t3rq", u3r, "c3q"),
    )

    def emit_colsum(oh):
        # bias: colsum of ssq+ssr via DR matmuls with all-ones stationary;
        # bias_sb holds bias/WSCALE (the yo stt adds it after descale)
        bias_ps = psum_b.tile([P, OW], FP32, tag="bias_ps",
                              name=f"bias_ps{oh}")
        n_cs = 2 * KP
        ci = 0
        for src in ("ssq", "ssr"):
            for kp in range(KP):
                nc.tensor.matmul(
                    bias_ps[:], ones_dr[:],
                    sssb[(src, oh)][:, 2 * kp:2 * kp + 2, :],
                    start=(ci == 0), stop=(ci == n_cs - 1),
                    perf_mode=mybir.MatmulPerfMode.DoubleRow)
                ci += 1
        nc.scalar.activation(out=bias_sb[oh][:], in_=bias_ps[:],
                             func=mybir.ActivationFunctionType.Copy,
                             scale=1.0 / WSCALE)
        if cfg["splits"]:
            nc.scalar.activation(out=bias_sc[oh][:], in_=bias_ps[:],
                                 func=mybir.ActivationFunctionType.Copy)

    def mm(po, u, cname, oh, kp, j, start, stop=False):
        nc.tensor.matmul(
            po[:],
            u[:, 2 * kp:2 * kp + 2, j * P:(j + 1) * P],
            wsb[(cname, oh)][:, 2 * kp:2 * kp + 2, :],
            start=start, stop=stop,
            perf_mode=mybir.MatmulPerfMode.DoubleRow)

    close_n = [0]

    def emit_close(po, oh, j, split=1):
        # yo = po/WSCALE + bias, alternating between the two late-phase-idle
        # elementwise engines so closes never serialize.  split>1 chops the
        # close into parallel column strips (tail-latency reduction for the
        # last groups).
        yo = yopool.tile([P, OW], BF16, tag="yo")
        step = OW // split
        for s in range(split):
            c = slice(s * step, (s + 1) * step)
            eng = yengs[close_n[0] % len(yengs)]
            close_n[0] += 1
            eng.scalar_tensor_tensor(out=yo[:, c], in0=po[:, c],
                                     scalar=1.0 / WSCALE,
                                     in1=bias_sb[oh][:, c],
                                     op0=MULT, op1=ADD)
            nc.sync.dma_start(
                out=y_d[j * P:(j + 1) * P,
                        oh * OW + s * step:oh * OW + (s + 1) * step],
                in_=yo[:, c])

    yengs = {"vector": (nc.vector,)}[cfg["yo_engine"]]

    # phase A (u-prep-gated): first NA groups of oh 0, ki-pair-major so PE
    # consumption paces with u production.  phase B (free-running): the
    # rest, group-major so groups close staggered and yo/DMA overlap PE.
    NA = min(cfg["wave_a"], NJ)
    NC_ = min(cfg["companions"], NJ - NA)  # q-only companion groups
    pos_a = {j: psum_o.tile([P, OW], FP32, tag="po", name=f"poA{j}")
             for j in range(NA + NC_)}
    # availability-ordered (pass-group, kp) interleave: q-passes stream off
    # DVE (fast), r-passes off the residual engine (slower).  Companion
    # groups join only the q-columns (their r-columns run at phase-B start,
    # filling early PE gaps without extra PSUM pressure later).
    qs, rs = PASSES[:3], PASSES[3:]
    sched = [(qs, int(c[1:])) if c[0] == "q" else (rs, int(c[1:]))
             for c in cfg["sched"].split()]
    sched = [(grp, kp) for grp, kp in sched if kp < KP]
    n_q = sum(len(g) for g, _ in sched if g is qs)
    n_r = sum(len(g) for g, _ in sched if g is rs)
    total = {j: (n_q + n_r if j < NA else n_q + len(rs) * KP)
             for j in range(NA + NC_)}
    done = {j: 0 for j in range(NA + NC_)}

    def mm_a(j, u, cname, kp):
        done[j] += 1
        mm(pos_a[j], u, cname, 0, kp, j, start=(done[j] == 1),
           stop=(done[j] == total[j]))

    # Split groups: output tiles whose kp0..KPH-1 contribution runs during
    # phase A in a rotating PSUM bank, parked to an SBUF partial (ACT copy,
    # bank freed) and merged at the final close.  Gives PE fill work while
    # the u streams pace phase A.
    all_groups = [(0, j) for j in range(NJ)]
    all_groups += [(1, j) for j in range(NJ)] if OH > 1 else []
    split_groups = ([g for g in all_groups[NA + NC_:]][:cfg["splits"]]
                    if KP >= 2 else [])
    KPH = max(1, KP // 2)
    partials = {}

    def emit_split_early(oh, j):
        po = psum_o.tile([P, OW], FP32, tag="po", name=f"poS{oh}_{j}")
        n = 0
        for kp in range(KPH):
            for pi, (_, u, cname) in enumerate(PASSES):
                n += 1
                mm(po, u, cname, oh, kp, j, start=(n == 1),
                   stop=(n == KPH * len(PASSES)))
        part = singles.tile([P, OW], BF16, name=f"part{oh}_{j}")
        nc.scalar.activation(out=part[:], in_=po[:],
                             func=mybir.ActivationFunctionType.Copy,
                             scale=1.0 / WSCALE)
        partials[(oh, j)] = part

    def emit_split_final(oh, j):
        po = psum_o.tile([P, OW], FP32, tag="po", name=f"poF{oh}_{j}")
        n = 0
        for kp in range(KPH, KP):
            for pi, (_, u, cname) in enumerate(PASSES):
                n += 1
                mm(po, u, cname, oh, kp, j, start=(n == 1))
        nc.tensor.matmul(po[:], e_row[:], bias_sc[oh][:], start=False,
                         stop=True)
        yo = yopool.tile([P, OW], BF16, tag="yo")
        nc.vector.scalar_tensor_tensor(
            out=yo[:], in0=po[:], scalar=1.0 / WSCALE,
            in1=partials[(oh, j)][:], op0=MULT, op1=ADD)
        nc.sync.dma_start(
            out=y_d[j * P:(j + 1) * P, oh * OW:(oh + 1) * OW], in_=yo[:])

    splits_iter = list(split_groups)
    for si, (grp, kp) in enumerate(sched):
        is_q = grp is qs
        for _, u, cname in grp:
            for j in range(NA + (NC_ if is_q else 0)):
                mm_a(j, u, cname, kp)
        if si == cfg["colsum_at"]:
            emit_colsum(0)
        if si >= cfg["split_from"] and splits_iter:
            oh, j = splits_iter.pop(0)
            emit_split_early(oh, j)
    if cfg["colsum_at"] >= len(sched):
        emit_colsum(0)
    while splits_iter:
        emit_split_early(*splits_iter.pop(0))
    # companions: finish their r-columns, then close everything
    for kp in range(KP):
        for _, u, cname in rs:
            for j in range(NA, NA + NC_):
                mm_a(j, u, cname, kp)
    for j in range(NA + NC_):
        emit_close(pos_a[j], 0, j)

    if OH > 1:
        emit_colsum(1)
    for oh, j in split_groups:
        emit_split_final(oh, j)
    for gi, (oh, j) in enumerate(all_groups[NA + NC_:]):
        if (oh, j) in partials:
            continue
        po = psum_o.tile([P, OW], FP32, tag="po", name=f"poB{oh}_{j}")
        for kp in range(KP):
            for pi, (_, u, cname) in enumerate(PASSES):
                mm(po, u, cname, oh, kp, j, start=(kp == 0 and pi == 0),
                   stop=(kp == KP - 1 and pi == len(PASSES) - 1))
        emit_close(po, oh, j)


W_PLANE_NAMES = ("c1q", "c2q", "c2r", "c3q", "c3r", "ssq", "ssr")


def build_nc(b_loc=B_FULL // N_CORES, i_dim=I_DIM, o_dim=O_DIM,
             n_cores=N_CORES, cfg=None):
    nc = bacc.Bacc("TRN2", target_bir_lowering=False, debug=False,
                   num_devices=n_cores)
    KI = i_dim // P
    xt_d = nc.dram_tensor("xt", [P, KI, b_loc], BF16,
                          kind="ExternalInput").ap()
    wplanes_d = {
        name: nc.dram_tensor(name, [P, KI, o_dim], FP8,
                             kind="ExternalInput").ap()
        for name in W_PLANE_NAMES
    }
    y_d = nc.dram_tensor("y", [b_loc, o_dim], BF16, kind="ExternalOutput").ap()
    with tile.TileContext(nc) as tc:
        _bessel_body(tc, y_d, xt_d, wplanes_d, b_loc, i_dim, o_dim, cfg=cfg)
    nc.compile()
    return nc


def prep_inputs(x, w, n_cores=N_CORES):
    """Host-side data prep: shard/permute x, fold + cast weight planes."""
    x = np.asarray(x, dtype=np.float32)
    w = np.asarray(w, dtype=np.float32)
    b_full, i_dim = x.shape
    o_dim = w.shape[1]
    KI = i_dim // P
    b_loc = b_full // n_cores

    # x^T permuted to [p, ki, b] (i = ki*P + p), cast bf16 (halves DMA)
    xt = np.ascontiguousarray(
        x.T.reshape(KI, P, b_full).transpose(1, 0, 2)).astype(BF16_NP)

    w64 = w.astype(np.float64)
    W0, W1, W2, W3 = (w64[..., d] for d in range(4))
    planes64 = {
        "c1": W1 + 3 * W2 + 6 * W3,
        "c2": W2 + 5 * W3,
        "c3": W3,
        "ss": W0 + W1 + W2 + W3,
    }

    def perm(a):  # [I, O] -> [p, ki, O]
        return np.ascontiguousarray(
            a.reshape(KI, P, o_dim).transpose(1, 0, 2))

    def to_fp8(a):  # saturating e4m3 cast (TRN max normal 240)
        return np.clip(a, -240.0, 240.0).astype(FP8_NP)

    out = {"c1q": perm(to_fp8(WSCALE * planes64["c1"]))}
    for name in ("c2", "c3", "ss"):
        hi64 = WSCALE * planes64[name]
        q = to_fp8(hi64)
        r = to_fp8(hi64 - q.astype(np.float64))
        out[name + "q"] = perm(q)
        out[name + "r"] = perm(r)

    in_maps = []
    for c in range(n_cores):
        m = {"xt": np.ascontiguousarray(
            xt[:, :, c * b_loc:(c + 1) * b_loc])}
        m.update(out)
        in_maps.append(m)
    return in_maps


_NC_CACHE = {}


def _get_nc():
    if "full" not in _NC_CACHE:
        _NC_CACHE["full"] = build_nc()
    return _NC_CACHE["full"]


def run_spmd(x, bessel_coeffs, trace=False, **kwargs):
    from concourse.bass_utils import run_bass_kernel_spmd

    nc = _get_nc()
    in_maps = prep_inputs(x, bessel_coeffs)
    res = run_bass_kernel_spmd(nc, in_maps, core_ids=list(range(N_CORES)),
                               trace=trace, **kwargs)
    y = np.concatenate(
        [np.asarray(r["y"]).astype(np.float32) for r in res.results], axis=0)
    return y, res


def kernel(x, bessel_coeffs):
    y, _ = run_spmd(x, bessel_coeffs)
    return y.astype(np.float32)


def _ref_np(x, w):
    t = np.tanh(np.asarray(x, dtype=np.float64))
    w = np.asarray(w, dtype=np.float64)
    basis = [np.ones_like(t), t + 1.0]
    for i in range(2, NDEG):
        basis.append((2 * i - 1) * t * basis[i - 1] + basis[i - 2])
    bz = np.stack(basis, axis=-1)
    return np.einsum("bid,iod->bo", bz, w)


def _selftest_sim(b_loc=256, i_dim=256, o_dim=1024):
    """CoreSim check on a small config exercising all loop paths."""
    from concourse.bass_interp import CoreSim

    nc = build_nc(b_loc=b_loc, i_dim=i_dim, o_dim=o_dim, n_cores=1)
    rng = np.random.default_rng(0)
    x = rng.standard_normal((b_loc, i_dim)).astype(np.float32)
    w = (rng.standard_normal((i_dim, o_dim, NDEG)) / (i_dim * NDEG)).astype(
        np.float32)
    in_maps = prep_inputs(x, w, n_cores=1)
    sim = CoreSim(nc)
    for name, arr in in_maps[0].items():
        sim.tensor(name)[:] = arr
    sim.simulate()
    y = np.array(sim.tensor("y")).astype(np.float64)
    ref = _ref_np(x, w)
    scale = np.abs(ref).max()
    err = np.abs(y - ref).max() / scale
    print(f"sim scale={scale:.4g} max_abs_rel_err={err:.4g}")
    assert err < 2e-2, err
    print("SIM OK")


def _timesim(cfg=None):
    from concourse.timeline_sim import TimelineSim

    nc = build_nc(cfg=cfg)
    t = TimelineSim(nc).simulate()
    print(f"TimelineSim: {t:.0f} ns")
    return t


if __name__ == "__main__":
    if "--sim" in sys.argv:
        _selftest_sim()
    if "--timesim" in sys.argv:
        _timesim()


# revision 60
# speedup vs baseline: 1.7207x; 1.0093x over previous
"""BesselKAN layer kernel for Trainium2 (8 NeuronCores, data-parallel batch).

reference math:
    t = tanh(x)                                   # [B, I]
    b0 = 1; b1 = 1+t; b2 = 1+3t+3t^2; b3 = 1+6t+15t^2+15t^3
    y[b,o] = sum_{i,d} b_d[b,i] * W[i,o,d]        # W = bessel_coeffs [I, O, 4]

Monomial rewrite (exact algebra):
    y = bias + t @ C1 + (3 t^2) @ C2 + (15 t^3) @ C3
    C1 = W1 + 3 W2 + 6 W3 ; C2 = W2 + 5 W3 ; C3 = W3
    bias_o = colsum(W0 + W1 + W2 + W3)

Device strategy (per core, 1024 batch rows):
  - All contraction matmuls run as fp8e4m3 DoubleRow (2x PE rate, 0.5
    cycles/column).  The t^2/t^3 terms carry most of the signal, so
    their operands are split hi/lo: u = q(u) + r, C = q(C) + r(C), and
    three DR passes per term (q@q, r@q, q@r) recover ~bf16 accuracy at
    1.5x the cost of one fp8 pass instead of 2x (bf16).  The t term is
    small enough for a single plain fp8 pass.
  - Weight-side planes are host-prepared (layout permute + monomial fold
    + 2^13 pre-scale + saturating fp8 casts): c1q, c2q/c2r, c3q/c3r,
    ssq/ssr.  The 8192x pre-scale (exact power of two) keeps the fp8
    residual planes out of the subnormal range; the final yo op
    descales by 1/8192.
  - x is host-transposed (and bf16-cast) so tanh lands directly in
    [i_part, b] layout - no PE transposes anywhere.  u-side pipeline per
    ki: tanh (ACT), 3t^2 = Square(sqrt3 * t) (ACT), 15t^3 (DVE stt),
    fp8 casts (DVE/GpSimd tensor_copy), residuals u - q(u) (DVE/GpSimd
    tensor_tensor subtract; GPSIMD cannot run TensorScalarPtr or touch
    PSUM on trn2).
  - bias: colsum of ssq+ssr via DR matmuls with an all-ones fp8
    stationary; added during the PSUM drain: yo = po/8192 + bias
    (DVE scalar_tensor_tensor, out bf16) -> DMA; host upcasts to f32.
  - emission: ki-pair-major over an "A" superwave of 7 PSUM groups
    whose columns are ordered by estimated operand availability
    (sched="auto"), so PE consumption paces the ACT/DVE/GpSimd
    elementwise streams; remaining groups run group-major afterwards,
    closing staggered so yo/DMA overlap PE.
"""

import sys
from contextlib import ExitStack

import numpy as np

if "/opt/trn_rl_repo" not in sys.path:
    sys.path.insert(0, "/opt/trn_rl_repo")

import ml_dtypes

import concourse.bass as bass
import concourse.tile as tile
from concourse import bacc, mybir
from concourse._compat import with_exitstack

P = 128
N_CORES = 8
B_FULL = 8192
I_DIM = 1024
O_DIM = 1024
NDEG = 4

FP32 = mybir.dt.float32
BF16 = mybir.dt.bfloat16
FP8 = mybir.dt.float8e4

SQRT3 = float(np.sqrt(3.0))
WSCALE = 8192.0  # weight-plane pre-scale (2^13, exact)

BF16_NP = ml_dtypes.bfloat16
FP8_NP = ml_dtypes.float8_e4m3

MULT = mybir.AluOpType.mult
ADD = mybir.AluOpType.add
SUB = mybir.AluOpType.subtract

DEFAULT_CFG = dict(
    xbufs=4,
    wbufs=4,
    yobufs=4,
    pbufs=8,
    wave_a=7,
    bias_shared_pool=True,
    nsplit=2,
    nsplit_kis=2,
    resid_engine="gpsimd",  # or "vector"
    yo_engine="vector",  # vector | scalar2 (gpsimd cannot read PSUM)
    sched="auto",
    w_chunk=True,
    colsum_at=99,
    companions=0,
    splits=0,
    split_from=3,
    tailpipe=0,
    w_order=("c1q0 c2q0 c3q0 c2r0 c3r0 c1q0 c2q0 c3q0 c2r0 c3r0 "
             "ssq0 ssr0 c1q1 c2q1 c3q1 c2r1 c3r1 ssq1 ssr1"),
)


@with_exitstack
def _bessel_body(ctx: ExitStack, tc: "tile.TileContext", y_d, xt_d, wplanes_d,
                 b_loc, i_dim, o_dim, cfg=None):
    """wplanes_d: dict name -> dram AP, each [P, KI, o_dim]:
    c1q(fp8) c2q c2r c3q c3r (fp8) ssq ssr (fp8)."""
    cfg = {**DEFAULT_CFG, **(cfg or {})}
    nc = tc.nc
    KI = i_dim // P
    KP = KI // 2
    NJ = b_loc // P
    OW = min(512, o_dim)
    OH = o_dim // OW
    resid = nc.gpsimd if cfg["resid_engine"] == "gpsimd" else nc.vector

    singles = ctx.enter_context(tc.tile_pool(name="singles", bufs=1))
    xpool = ctx.enter_context(tc.tile_pool(name="xpool", bufs=cfg["xbufs"]))
    wpool = ctx.enter_context(tc.tile_pool(name="wpool", bufs=cfg["wbufs"]))
    yopool = ctx.enter_context(tc.tile_pool(name="yopool", bufs=cfg["yobufs"]))
    psum_o = ctx.enter_context(
        tc.tile_pool(name="psum_o", bufs=cfg["pbufs"], space="PSUM"))
    psum_b = psum_o if cfg["bias_shared_pool"] else ctx.enter_context(
        tc.tile_pool(name="psum_b", bufs=1, space="PSUM"))

    # constants
    ones_dr = singles.tile([P, 2, P], FP8, name="ones_dr")
    nc.vector.memset(ones_dr[:], 1.0)
    e_row = singles.tile([P, P], BF16, name="e_row")
    nc.vector.memset(e_row[:], 0.0)
    nc.vector.memset(e_row[0:1, :], 1.0)

    # u-side persistent tensors, [i_part, ki, b]
    u1b = singles.tile([P, KI, b_loc], BF16, name="u1b")
    u1q = singles.tile([P, KI, b_loc], FP8, name="u1q")
    u2b = singles.tile([P, KI, b_loc], BF16, name="u2b")
    u2q = singles.tile([P, KI, b_loc], FP8, name="u2q")
    u2r = singles.tile([P, KI, b_loc], FP8, name="u2r")
    u3b = singles.tile([P, KI, b_loc], BF16, name="u3b")
    u3q = singles.tile([P, KI, b_loc], FP8, name="u3q")
    u3r = singles.tile([P, KI, b_loc], FP8, name="u3r")

    # weight-side persistent fp8 tiles per output half
    W_NAMES = ("c1q", "c2q", "c2r", "c3q", "c3r")
    wsb = {(n, oh): singles.tile([P, KI, OW], FP8, name=f"{n}_{oh}")
           for n in W_NAMES for oh in range(OH)}
    sssb = {(n, oh): singles.tile([P, KI, OW], FP8, name=f"{n}_{oh}")
            for n in ("ssq", "ssr") for oh in range(OH)}
    bias_sb = [singles.tile([P, OW], BF16, name=f"bias_sb{oh}")
               for oh in range(OH)]
    bias_sc = [singles.tile([P, OW], BF16, name=f"bias_sc{oh}")
               for oh in range(OH)]

    def emit_u_pair(kp, nsplit=1, wtake=()):
        # Emit both kis of a contraction pair with ops grouped by matmul-pass
        # consumption priority (u1q -> u2q -> u3q -> residuals) so each
        # engine's in-order queue produces pair-complete tensors asap.
        # nsplit>1 additionally halves the b-range per op for shorter chain
        # latency at kernel startup.
        kis = [2 * kp, 2 * kp + 1][:max(1, KI - 2 * kp)]
        xts = {}
        for ki in kis:
            xts[ki] = xpool.tile([P, b_loc], BF16, tag="x_t", name=f"x_t{ki}")
        step = b_loc // nsplit
        for s in range(nsplit):
            for _ in range(wtake[s] if s < len(wtake) else 0):
                if wq:
                    emit_wdma(*wq.pop(0))
            bsl = slice(s * step, (s + 1) * step)

            def sl(ki):
                return (slice(None), ki, bsl)

            for ki in kis:
                nc.sync.dma_start(out=xts[ki][:, bsl], in_=xt_d[:, ki, bsl])
                nc.scalar.activation(out=u1b[sl(ki)], in_=xts[ki][:, bsl],
                                     func=mybir.ActivationFunctionType.Tanh)
            for ki in kis:
                nc.vector.tensor_copy(out=u1q[sl(ki)], in_=u1b[sl(ki)])
                nc.scalar.activation(out=u2b[sl(ki)], in_=u1b[sl(ki)],
                                     func=mybir.ActivationFunctionType.Square,
                                     scale=SQRT3)
            for ki in kis:
                nc.gpsimd.tensor_copy(out=u2q[sl(ki)], in_=u2b[sl(ki)])
            for ki in kis:
                nc.vector.scalar_tensor_tensor(
                    out=u3b[sl(ki)], in0=u1b[sl(ki)], scalar=5.0,
                    in1=u2b[sl(ki)], op0=MULT, op1=MULT)
                nc.vector.tensor_copy(out=u3q[sl(ki)], in_=u3b[sl(ki)])
            for ki in kis:
                nc.vector.tensor_tensor(out=u2r[sl(ki)], in0=u2b[sl(ki)],
                                        in1=u2q[sl(ki)], op=SUB)
            for ki in kis:
                nc.gpsimd.tensor_tensor(out=u3r[sl(ki)], in0=u3b[sl(ki)],
                                        in1=u3q[sl(ki)], op=SUB)

    def emit_wdma(name, oh, kis=None):
        dst = sssb[(name, oh)] if name in ("ssq", "ssr") else wsb[(name, oh)]
        kis = kis or (0, KI)
        nc.sync.dma_start(
            out=dst[:, kis[0]:kis[1], :],
            in_=wplanes_d[name][:, kis[0]:kis[1], oh * OW:(oh + 1) * OW])

    # ---- emission: u-prep interleaved with W DMAs (program order ~ priority).
    # q-planes stream first (they gate the early matmul passes), residual
    # planes next, ss planes last (bias is only needed at group close).
    # The first c3q/c2q chunks are split so the first passes' operands land
    # with the first x tiles.
    kc = min(2, KI)
    W_CHUNKED = {"c1q", "c2q", "c3q", "c2r", "c3r"}

    def worder():
        # cfg string: space-separated "<plane><oh>" tokens; chunked planes
        # expand to (0,kc) + (kc,KI) at their first/second occurrence
        seen = set()
        out = []
        for tok in cfg["w_order"].split():
            n, oh = tok[:3], int(tok[3])
            if oh >= OH:
                continue
            if n in W_CHUNKED and cfg["w_chunk"] and oh == 0:
                if (n, oh) not in seen:
                    out.append((n, oh, (0, kc)))
                    seen.add((n, oh))
                else:
                    out.append((n, oh, (kc, KI)))
            elif (n, oh) not in seen:
                out.append((n, oh, None))
                seen.add((n, oh))
        emitted = {}
        for n, oh, k in out:
            lo, hi = k if k else (0, KI)
            emitted[(n, oh)] = max(emitted.get((n, oh), 0), hi)
        need = [(n, oh) for n in
                ("c1q", "c2q", "c2r", "c3q", "c3r", "ssq", "ssr")
                for oh in range(OH)]
        for n, oh in need:
            hi = emitted.get((n, oh), 0)
            if hi < KI:
                out.append((n, oh, (hi, KI)))
        return [e for e in out if e[2] is None or e[2][0] < e[2][1]]

    wq = worder()
    for kp in range(KP):
        if kp == 0:
            emit_u_pair(kp, nsplit=cfg["nsplit"], wtake=(0, 2))
            take = 2
        else:
            emit_u_pair(kp)
            take = 2
        for _ in range(take):
            if wq:
                emit_wdma(*wq.pop(0))
    while wq:
        emit_wdma(*wq.pop(0))

    # (u fp8 tensor, weight plane name) passes per accumulation group,
    # ordered by when the operands become available (q before residual)
    PASSES = (
        ("t1qq", u1q, "c1q"), ("t2qq", u2q, "c2q"), ("t3qq", u3q, "c3q"),
        ("t2qr", u2q, "c2r"), ("t3qr", u3q, "c3r"),
        ("t2rq", u2r, "c2q"), ("t3rq", u3r, "c3q"),
    )

    def emit_colsum(oh):
        # bias: colsum of ssq+ssr via DR matmuls with all-ones stationary;
        # bias_sb holds bias/WSCALE (the yo stt adds it after descale)
        bias_ps = psum_b.tile([P, OW], FP32, tag="po",
                              name=f"bias_ps{oh}")
        n_cs = 2 * KP
        ci = 0
        for src in ("ssq", "ssr"):
            for kp in range(KP):
                nc.tensor.matmul(
                    bias_ps[:], ones_dr[:],
                    sssb[(src, oh)][:, 2 * kp:2 * kp + 2, :],
                    start=(ci == 0), stop=(ci == n_cs - 1),
                    perf_mode=mybir.MatmulPerfMode.DoubleRow)
                ci += 1
        nc.scalar.activation(out=bias_sb[oh][:], in_=bias_ps[:],
                             func=mybir.ActivationFunctionType.Copy,
                             scale=1.0 / WSCALE)
        if cfg["splits"]:
            nc.scalar.activation(out=bias_sc[oh][:], in_=bias_ps[:],
                                 func=mybir.ActivationFunctionType.Copy)

    def mm(po, u, cname, oh, kp, j, start, stop=False):
        nc.tensor.matmul(
            po[:],
            u[:, 2 * kp:2 * kp + 2, j * P:(j + 1) * P],
            wsb[(cname, oh)][:, 2 * kp:2 * kp + 2, :],
            start=start, stop=stop,
            perf_mode=mybir.MatmulPerfMode.DoubleRow)

    close_n = [0]

    def emit_close(po, oh, j, split=1):
        # yo = po/WSCALE + bias, alternating between the two late-phase-idle
        # elementwise engines so closes never serialize.  split>1 chops the
        # close into parallel column strips (tail-latency reduction for the
        # last groups).
        yo = yopool.tile([P, OW], BF16, tag="yo")
        step = OW // split
        for s in range(split):
            c = slice(s * step, (s + 1) * step)
            eng = yengs[close_n[0] % len(yengs)]
            close_n[0] += 1
            eng.scalar_tensor_tensor(out=yo[:, c], in0=po[:, c],
                                     scalar=1.0 / WSCALE,
                                     in1=bias_sb[oh][:, c],
                                     op0=MULT, op1=ADD)
            nc.sync.dma_start(
                out=y_d[j * P:(j + 1) * P,
                        oh * OW + s * step:oh * OW + (s + 1) * step],
                in_=yo[:, c])

    yengs = {"vector": (nc.vector,)}[cfg["yo_engine"]]

    # phase A (u-prep-gated): first NA groups of oh 0, ki-pair-major so PE
    # consumption paces with u production.  phase B (free-running): the
    # rest, group-major so groups close staggered and yo/DMA overlap PE.
    NA = min(cfg["wave_a"], NJ)
    NC_ = min(cfg["companions"], NJ - NA)  # q-only companion groups
    pos_a = {j: psum_o.tile([P, OW], FP32, tag="po", name=f"poA{j}")
             for j in range(NA + NC_)}
    # availability-ordered (pass-group, kp) interleave: q-passes stream off
    # DVE (fast), r-passes off the residual engine (slower).  Companion
    # groups join only the q-columns (their r-columns run at phase-B start,
    # filling early PE gaps without extra PSUM pressure later).
    qs, rs = PASSES[:3], PASSES[3:]
    if cfg["sched"] == "auto":
        # sort single-pass columns by estimated operand availability
        # (pair-rate ~6.9us on DVE/Pool; offsets from per-pair queue order)
        OFFS = {"t1qq": 1.2, "t2qq": 2.9, "t3qq": 4.7, "t2qr": 2.9,
                "t3qr": 4.7, "t2rq": 6.9, "t3rq": 6.9}
        PLANE = {"t1qq": 2.0, "t2qq": 1.0, "t3qq": 3.0, "t2qr": 5.0,
                 "t3qr": 6.0, "t2rq": 1.0, "t3rq": 3.0}
        cols = []
        for pi, p in enumerate(PASSES):
            for kp in range(KP):
                est = max(4.0 + 6.9 * kp + OFFS[p[0]], 2.0 + PLANE[p[0]])
                cols.append((est, kp, pi, p))
        cols.sort(key=lambda c: (c[0], c[1]))
        sched = [((p,), kp) for _, kp, _, p in cols]
    else:
        sched = [(qs, int(c[1:])) if c[0] == "q" else (rs, int(c[1:]))
                 for c in cfg["sched"].split()]
        sched = [(grp, kp) for grp, kp in sched if kp < KP]
    def grp_is_q(g):
        return all(p[0].endswith("qq") or p[0].endswith("qr") for p in g) \
            and g is not rs
    n_q = sum(len(g) for g, _ in sched if grp_is_q(g))
    n_r = sum(len(g) for g, _ in sched if not grp_is_q(g))
    total = {j: (n_q + n_r if j < NA else n_q + len(rs) * KP)
             for j in range(NA + NC_)}
    done = {j: 0 for j in range(NA + NC_)}

    def mm_a(j, u, cname, kp):
        done[j] += 1
        mm(pos_a[j], u, cname, 0, kp, j, start=(done[j] == 1),
           stop=(done[j] == total[j]))

    # Split groups: output tiles whose kp0..KPH-1 contribution runs during
    # phase A in a rotating PSUM bank, parked to an SBUF partial (ACT copy,
    # bank freed) and merged at the final close.  Gives PE fill work while
    # the u streams pace phase A.
    all_groups = [(0, j) for j in range(NJ)]
    all_groups += [(1, j) for j in range(NJ)] if OH > 1 else []
    split_groups = ([g for g in all_groups[NA + NC_:]][:cfg["splits"]]
                    if KP >= 2 else [])
    KPH = max(1, KP // 2)
    partials = {}

    def emit_split_early(oh, j):
        po = psum_o.tile([P, OW], FP32, tag="po", name=f"poS{oh}_{j}")
        n = 0
        for kp in range(KPH):
            for pi, (_, u, cname) in enumerate(PASSES):
                n += 1
                mm(po, u, cname, oh, kp, j, start=(n == 1),
                   stop=(n == KPH * len(PASSES)))
        part = singles.tile([P, OW], BF16, name=f"part{oh}_{j}")
        nc.scalar.activation(out=part[:], in_=po[:],
                             func=mybir.ActivationFunctionType.Copy,
                             scale=1.0 / WSCALE)
        partials[(oh, j)] = part

    def emit_split_final(oh, j):
        po = psum_o.tile([P, OW], FP32, tag="po", name=f"poF{oh}_{j}")
        n = 0
        for kp in range(KPH, KP):
            for pi, (_, u, cname) in enumerate(PASSES):
                n += 1
                mm(po, u, cname, oh, kp, j, start=(n == 1))
        nc.tensor.matmul(po[:], e_row[:], bias_sc[oh][:], start=False,
                         stop=True)
        yo = yopool.tile([P, OW], BF16, tag="yo")
        nc.vector.scalar_tensor_tensor(
            out=yo[:], in0=po[:], scalar=1.0 / WSCALE,
            in1=partials[(oh, j)][:], op0=MULT, op1=ADD)
        nc.sync.dma_start(
            out=y_d[j * P:(j + 1) * P, oh * OW:(oh + 1) * OW], in_=yo[:])

    # tail-pipeline: the first non-A group's columns that don't depend on
    # the last-arriving residuals run right before phase A's final columns,
    # filling the stream-tail PE gap (uses the one spare PSUM buffer)
    b0 = all_groups[NA + NC_] if (cfg["tailpipe"] and KP >= 2
                                  and len(all_groups) > NA + NC_
                                  and not split_groups) else None
    tp_si = max(0, len(sched) - cfg["tailpipe"]) if b0 else None
    po_b0 = None
    nb0 = 0

    splits_iter = list(split_groups)
    for si, (grp, kp) in enumerate(sched):
        if b0 is not None and si == tp_si:
            po_b0 = psum_o.tile([P, OW], FP32, tag="po",
                                name=f"poTP{b0[0]}_{b0[1]}")
            for grp2, kp2 in sched[:tp_si]:
                for _, u2, cn2 in grp2:
                    nb0 += 1
                    mm(po_b0, u2, cn2, b0[0], kp2, b0[1], start=(nb0 == 1))
        is_q = grp_is_q(grp)
        for _, u, cname in grp:
            for j in range(NA + (NC_ if is_q else 0)):
                mm_a(j, u, cname, kp)
        if si == cfg["colsum_at"]:
            emit_colsum(0)
        if si >= cfg["split_from"] and splits_iter:
            oh, j = splits_iter.pop(0)
            emit_split_early(oh, j)
    if cfg["colsum_at"] >= len(sched):
        emit_colsum(0)
    while splits_iter:
        emit_split_early(*splits_iter.pop(0))
    # companions: finish their r-columns, then close everything
    for kp in range(KP):
        for _, u, cname in rs:
            for j in range(NA, NA + NC_):
                mm_a(j, u, cname, kp)
    if b0 is not None:
        n_total = sum(len(g) for g, _ in sched)
        for grp2, kp2 in sched[tp_si:]:
            for _, u2, cn2 in grp2:
                nb0 += 1
                mm(po_b0, u2, cn2, b0[0], kp2, b0[1], start=False,
                   stop=(nb0 == n_total))
    for j in range(NA + NC_):
        emit_close(pos_a[j], 0, j)
    if b0 is not None:
        emit_close(po_b0, b0[0], b0[1])

    if OH > 1:
        emit_colsum(1)
    for oh, j in split_groups:
        emit_split_final(oh, j)
    for gi, (oh, j) in enumerate(all_groups[NA + NC_:]):
        if (oh, j) in partials or (b0 is not None and (oh, j) == b0):
            continue
        po = psum_o.tile([P, OW], FP32, tag="po", name=f"poB{oh}_{j}")
        for kp in range(KP):
            for pi, (_, u, cname) in enumerate(PASSES):
                mm(po, u, cname, oh, kp, j, start=(kp == 0 and pi == 0),
                   stop=(kp == KP - 1 and pi == len(PASSES) - 1))
        emit_close(po, oh, j)


W_PLANE_NAMES = ("c1q", "c2q", "c2r", "c3q", "c3r", "ssq", "ssr")


def build_nc(b_loc=B_FULL // N_CORES, i_dim=I_DIM, o_dim=O_DIM,
             n_cores=N_CORES, cfg=None):
    nc = bacc.Bacc("TRN2", target_bir_lowering=False, debug=False,
                   num_devices=n_cores)
    KI = i_dim // P
    xt_d = nc.dram_tensor("xt", [P, KI, b_loc], BF16,
                          kind="ExternalInput").ap()
    wplanes_d = {
        name: nc.dram_tensor(name, [P, KI, o_dim], FP8,
                             kind="ExternalInput").ap()
        for name in W_PLANE_NAMES
    }
    y_d = nc.dram_tensor("y", [b_loc, o_dim], BF16, kind="ExternalOutput").ap()
    with tile.TileContext(nc) as tc:
        _bessel_body(tc, y_d, xt_d, wplanes_d, b_loc, i_dim, o_dim, cfg=cfg)
    nc.compile()
    return nc


def prep_inputs(x, w, n_cores=N_CORES):
    """Host-side data prep: shard/permute x, fold + cast weight planes."""
    x = np.asarray(x, dtype=np.float32)
    w = np.asarray(w, dtype=np.float32)
    b_full, i_dim = x.shape
    o_dim = w.shape[1]
    KI = i_dim // P
    b_loc = b_full // n_cores

    # x^T permuted to [p, ki, b] (i = ki*P + p), cast bf16 (halves DMA)
    xt = np.ascontiguousarray(
        x.T.reshape(KI, P, b_full).transpose(1, 0, 2)).astype(BF16_NP)

    w64 = w.astype(np.float64)
    W0, W1, W2, W3 = (w64[..., d] for d in range(4))
    planes64 = {
        "c1": W1 + 3 * W2 + 6 * W3,
        "c2": W2 + 5 * W3,
        "c3": W3,
        "ss": W0 + W1 + W2 + W3,
    }

    def perm(a):  # [I, O] -> [p, ki, O]
        return np.ascontiguousarray(
            a.reshape(KI, P, o_dim).transpose(1, 0, 2))

    def to_fp8(a):  # saturating e4m3 cast (TRN max normal 240)
        return np.clip(a, -240.0, 240.0).astype(FP8_NP)

    out = {"c1q": perm(to_fp8(WSCALE * planes64["c1"]))}
    for name in ("c2", "c3", "ss"):
        hi64 = WSCALE * planes64[name]
        q = to_fp8(hi64)
        r = to_fp8(hi64 - q.astype(np.float64))
        out[name + "q"] = perm(q)
        out[name + "r"] = perm(r)

    in_maps = []
    for c in range(n_cores):
        m = {"xt": np.ascontiguousarray(
            xt[:, :, c * b_loc:(c + 1) * b_loc])}
        m.update(out)
        in_maps.append(m)
    return in_maps


_NC_CACHE = {}


def _get_nc():
    if "full" not in _NC_CACHE:
        _NC_CACHE["full"] = build_nc()
    return _NC_CACHE["full"]


def run_spmd(x, bessel_coeffs, trace=False, **kwargs):
    from concourse.bass_utils import run_bass_kernel_spmd

    nc = _get_nc()
    in_maps = prep_inputs(x, bessel_coeffs)
    res = run_bass_kernel_spmd(nc, in_maps, core_ids=list(range(N_CORES)),
                               trace=trace, **kwargs)
    y = np.concatenate(
        [np.asarray(r["y"]).astype(np.float32) for r in res.results], axis=0)
    return y, res


def kernel(x, bessel_coeffs):
    y, _ = run_spmd(x, bessel_coeffs)
    return y.astype(np.float32)


def _ref_np(x, w):
    t = np.tanh(np.asarray(x, dtype=np.float64))
    w = np.asarray(w, dtype=np.float64)
    basis = [np.ones_like(t), t + 1.0]
    for i in range(2, NDEG):
        basis.append((2 * i - 1) * t * basis[i - 1] + basis[i - 2])
    bz = np.stack(basis, axis=-1)
    return np.einsum("bid,iod->bo", bz, w)


def _selftest_sim(b_loc=256, i_dim=256, o_dim=1024):
    """CoreSim check on a small config exercising all loop paths."""
    from concourse.bass_interp import CoreSim

    nc = build_nc(b_loc=b_loc, i_dim=i_dim, o_dim=o_dim, n_cores=1)
    rng = np.random.default_rng(0)
    x = rng.standard_normal((b_loc, i_dim)).astype(np.float32)
    w = (rng.standard_normal((i_dim, o_dim, NDEG)) / (i_dim * NDEG)).astype(
        np.float32)
    in_maps = prep_inputs(x, w, n_cores=1)
    sim = CoreSim(nc)
    for name, arr in in_maps[0].items():
        sim.tensor(name)[:] = arr
    sim.simulate()
    y = np.array(sim.tensor("y")).astype(np.float64)
    ref = _ref_np(x, w)
    scale = np.abs(ref).max()
    err = np.abs(y - ref).max() / scale
    print(f"sim scale={scale:.4g} max_abs_rel_err={err:.4g}")
    assert err < 2e-2, err
    print("SIM OK")


def _timesim(cfg=None):
    from concourse.timeline_sim import TimelineSim

    nc = build_nc(cfg=cfg)
    t = TimelineSim(nc).simulate()
    print(f"TimelineSim: {t:.0f} ns")
    return t


if __name__ == "__main__":
    if "--sim" in sys.argv:
        _selftest_sim()
    if "--timesim" in sys.argv:
        _timesim()


# revision 64
# speedup vs baseline: 1.7888x; 1.0396x over previous
"""BesselKAN layer kernel for Trainium2 (8 NeuronCores, data-parallel batch).

reference math:
    t = tanh(x)                                   # [B, I]
    b0 = 1; b1 = 1+t; b2 = 1+3t+3t^2; b3 = 1+6t+15t^2+15t^3
    y[b,o] = sum_{i,d} b_d[b,i] * W[i,o,d]        # W = bessel_coeffs [I, O, 4]

Monomial rewrite (exact algebra):
    y = bias + t @ C1 + (3 t^2) @ C2 + (15 t^3) @ C3
    C1 = W1 + 3 W2 + 6 W3 ; C2 = W2 + 5 W3 ; C3 = W3
    bias_o = colsum(W0 + W1 + W2 + W3)

Device strategy (per core, 1024 batch rows):
  - All contraction matmuls run as fp8e4m3 DoubleRow (2x PE rate, 0.5
    cycles/column).  The t^2/t^3 terms carry most of the signal, so
    their operands are split hi/lo: u = q(u) + r, C = q(C) + r(C), and
    three DR passes per term (q@q, r@q, q@r) recover ~bf16 accuracy at
    1.5x the cost of one fp8 pass instead of 2x (bf16).  The t term is
    small enough for a single plain fp8 pass.
  - Weight-side planes are host-prepared (layout permute + monomial fold
    + 2^13 pre-scale + saturating fp8 casts): c1q, c2q/c2r, c3q/c3r,
    ssq/ssr.  The 8192x pre-scale (exact power of two) keeps the fp8
    residual planes out of the subnormal range; the final yo op
    descales by 1/8192.
  - x is host-transposed (and bf16-cast) so tanh lands directly in
    [i_part, b] layout - no PE transposes anywhere.  u-side pipeline per
    ki: tanh (ACT), 3t^2 = Square(sqrt3 * t) (ACT), 15t^3 (DVE stt),
    fp8 casts (DVE/GpSimd tensor_copy), residuals u - q(u) (DVE/GpSimd
    tensor_tensor subtract; GPSIMD cannot run TensorScalarPtr or touch
    PSUM on trn2).
  - bias: colsum of ssq+ssr via DR matmuls with an all-ones fp8
    stationary; added during the PSUM drain: yo = po/8192 + bias
    (DVE scalar_tensor_tensor, out bf16) -> DMA; host upcasts to f32.
  - emission: ki-pair-major over an "A" superwave of 7 PSUM groups
    whose columns are ordered by estimated operand availability
    (sched="auto"), so PE consumption paces the ACT/DVE/GpSimd
    elementwise streams; remaining groups run group-major afterwards,
    closing staggered so yo/DMA overlap PE.
"""

import sys
from contextlib import ExitStack

import numpy as np

if "/opt/trn_rl_repo" not in sys.path:
    sys.path.insert(0, "/opt/trn_rl_repo")

import ml_dtypes

import concourse.bass as bass
import concourse.tile as tile
from concourse import bacc, mybir
from concourse._compat import with_exitstack

P = 128
N_CORES = 8
B_FULL = 8192
I_DIM = 1024
O_DIM = 1024
NDEG = 4

FP32 = mybir.dt.float32
BF16 = mybir.dt.bfloat16
FP8 = mybir.dt.float8e4

SQRT3 = float(np.sqrt(3.0))
WSCALE = 8192.0  # weight-plane pre-scale (2^13, exact)

BF16_NP = ml_dtypes.bfloat16
FP8_NP = ml_dtypes.float8_e4m3

MULT = mybir.AluOpType.mult
ADD = mybir.AluOpType.add
SUB = mybir.AluOpType.subtract

DEFAULT_CFG = dict(
    xbufs=4,
    wbufs=4,
    yobufs=4,
    pbufs=8,
    wave_a=7,
    bias_shared_pool=True,
    nsplit=1,
    nsplit_kis=2,
    resid_engine="gpsimd",  # or "vector"
    yo_engine="vector",  # vector | scalar2 (gpsimd cannot read PSUM)
    sched="auto",
    w_chunk=True,
    colsum_at=99,
    companions=0,
    splits=0,
    split_from=3,
    tailpipe=0,
    w_order=("c1q0 c2q0 c3q0 c2r0 c3r0 c1q0 c2q0 c3q0 c2r0 c3r0 "
             "ssq0 ssr0 c1q1 c2q1 c3q1 c2r1 c3r1 ssq1 ssr1"),
)


@with_exitstack
def _bessel_body(ctx: ExitStack, tc: "tile.TileContext", y_d, xt_d, wplanes_d,
                 b_loc, i_dim, o_dim, cfg=None):
    """wplanes_d: dict name -> dram AP, each [P, KI, o_dim]:
    c1q(fp8) c2q c2r c3q c3r (fp8) ssq ssr (fp8)."""
    cfg = {**DEFAULT_CFG, **(cfg or {})}
    nc = tc.nc
    KI = i_dim // P
    KP = KI // 2
    NJ = b_loc // P
    OW = min(512, o_dim)
    OH = o_dim // OW
    resid = nc.gpsimd if cfg["resid_engine"] == "gpsimd" else nc.vector

    singles = ctx.enter_context(tc.tile_pool(name="singles", bufs=1))
    xpool = ctx.enter_context(tc.tile_pool(name="xpool", bufs=cfg["xbufs"]))
    wpool = ctx.enter_context(tc.tile_pool(name="wpool", bufs=cfg["wbufs"]))
    yopool = ctx.enter_context(tc.tile_pool(name="yopool", bufs=cfg["yobufs"]))
    psum_o = ctx.enter_context(
        tc.tile_pool(name="psum_o", bufs=cfg["pbufs"], space="PSUM"))
    psum_b = psum_o if cfg["bias_shared_pool"] else ctx.enter_context(
        tc.tile_pool(name="psum_b", bufs=1, space="PSUM"))

    # constants
    ones_dr = singles.tile([P, 2, P], FP8, name="ones_dr")
    nc.vector.memset(ones_dr[:], 1.0)
    e_row = singles.tile([P, P], BF16, name="e_row")
    nc.vector.memset(e_row[:], 0.0)
    nc.vector.memset(e_row[0:1, :], 1.0)

    # u-side persistent tensors, [i_part, ki, b]
    u1b = singles.tile([P, KI, b_loc], BF16, name="u1b")
    u1q = singles.tile([P, KI, b_loc], FP8, name="u1q")
    u2b = singles.tile([P, KI, b_loc], BF16, name="u2b")
    u2q = singles.tile([P, KI, b_loc], FP8, name="u2q")
    u2r = singles.tile([P, KI, b_loc], FP8, name="u2r")
    u3b = singles.tile([P, KI, b_loc], BF16, name="u3b")
    u3q = singles.tile([P, KI, b_loc], FP8, name="u3q")
    u3r = singles.tile([P, KI, b_loc], FP8, name="u3r")

    # weight-side persistent fp8 tiles per output half
    W_NAMES = ("c1q", "c2q", "c2r", "c3q", "c3r")
    wsb = {(n, oh): singles.tile([P, KI, OW], FP8, name=f"{n}_{oh}")
           for n in W_NAMES for oh in range(OH)}
    sssb = {(n, oh): singles.tile([P, KI, OW], FP8, name=f"{n}_{oh}")
            for n in ("ssq", "ssr") for oh in range(OH)}
    bias_sb = [singles.tile([P, OW], BF16, name=f"bias_sb{oh}")
               for oh in range(OH)]
    bias_sc = [singles.tile([P, OW], BF16, name=f"bias_sc{oh}")
               for oh in range(OH)]

    def emit_u_pair(kp, nsplit=1, wtake=()):
        # Emit both kis of a contraction pair with ops grouped by matmul-pass
        # consumption priority (u1q -> u2q -> u3q -> residuals) so each
        # engine's in-order queue produces pair-complete tensors asap.
        # nsplit>1 additionally halves the b-range per op for shorter chain
        # latency at kernel startup.
        kis = [2 * kp, 2 * kp + 1][:max(1, KI - 2 * kp)]
        xts = {}
        for ki in kis:
            xts[ki] = xpool.tile([P, b_loc], BF16, tag="x_t", name=f"x_t{ki}")
        step = b_loc // nsplit
        for s in range(nsplit):
            for _ in range(wtake[s] if s < len(wtake) else 0):
                if wq:
                    emit_wdma(*wq.pop(0))
            bsl = slice(s * step, (s + 1) * step)

            def sl(ki):
                return (slice(None), ki, bsl)

            for ki in kis:
                nc.sync.dma_start(out=xts[ki][:, bsl], in_=xt_d[:, ki, bsl])
                nc.scalar.activation(out=u1b[sl(ki)], in_=xts[ki][:, bsl],
                                     func=mybir.ActivationFunctionType.Tanh)
            for ki in kis:
                nc.scalar.activation(out=u2b[sl(ki)], in_=u1b[sl(ki)],
                                     func=mybir.ActivationFunctionType.Square,
                                     scale=SQRT3)
            for ci, ki in enumerate(kis):
                eng = nc.vector if ci % 2 == 0 else nc.gpsimd
                eng.tensor_copy(out=u2q[sl(ki)], in_=u2b[sl(ki)])
            for ki in kis:
                nc.scalar.activation(out=u1q[sl(ki)], in_=xts[ki][:, bsl],
                                     func=mybir.ActivationFunctionType.Tanh)
            for ki in kis:
                nc.vector.scalar_tensor_tensor(
                    out=u3b[sl(ki)], in0=u1b[sl(ki)], scalar=5.0,
                    in1=u2b[sl(ki)], op0=MULT, op1=MULT)
                nc.vector.tensor_copy(out=u3q[sl(ki)], in_=u3b[sl(ki)])
            for ki in kis:
                nc.vector.tensor_tensor(out=u2r[sl(ki)], in0=u2b[sl(ki)],
                                        in1=u2q[sl(ki)], op=SUB)
            for ki in kis:
                nc.gpsimd.tensor_tensor(out=u3r[sl(ki)], in0=u3b[sl(ki)],
                                        in1=u3q[sl(ki)], op=SUB)

    def emit_wdma(name, oh, kis=None):
        dst = sssb[(name, oh)] if name in ("ssq", "ssr") else wsb[(name, oh)]
        kis = kis or (0, KI)
        nc.sync.dma_start(
            out=dst[:, kis[0]:kis[1], :],
            in_=wplanes_d[name][:, kis[0]:kis[1], oh * OW:(oh + 1) * OW])

    # ---- emission: u-prep interleaved with W DMAs (program order ~ priority).
    # q-planes stream first (they gate the early matmul passes), residual
    # planes next, ss planes last (bias is only needed at group close).
    # The first c3q/c2q chunks are split so the first passes' operands land
    # with the first x tiles.
    kc = min(2, KI)
    W_CHUNKED = {"c1q", "c2q", "c3q", "c2r", "c3r"}

    def worder():
        # cfg string: space-separated "<plane><oh>" tokens; chunked planes
        # expand to (0,kc) + (kc,KI) at their first/second occurrence
        seen = set()
        out = []
        for tok in cfg["w_order"].split():
            n, oh = tok[:3], int(tok[3])
            if oh >= OH:
                continue
            if n in W_CHUNKED and cfg["w_chunk"] and oh == 0:
                if (n, oh) not in seen:
                    out.append((n, oh, (0, kc)))
                    seen.add((n, oh))
                else:
                    out.append((n, oh, (kc, KI)))
            elif (n, oh) not in seen:
                out.append((n, oh, None))
                seen.add((n, oh))
        emitted = {}
        for n, oh, k in out:
            lo, hi = k if k else (0, KI)
            emitted[(n, oh)] = max(emitted.get((n, oh), 0), hi)
        need = [(n, oh) for n in
                ("c1q", "c2q", "c2r", "c3q", "c3r", "ssq", "ssr")
                for oh in range(OH)]
        for n, oh in need:
            hi = emitted.get((n, oh), 0)
            if hi < KI:
                out.append((n, oh, (hi, KI)))
        return [e for e in out if e[2] is None or e[2][0] < e[2][1]]

    wq = worder()
    for kp in range(KP):
        if kp == 0:
            emit_u_pair(kp, nsplit=cfg["nsplit"], wtake=(0, 2))
            take = 2
        else:
            emit_u_pair(kp)
            take = 2
        for _ in range(take):
            if wq:
                emit_wdma(*wq.pop(0))
    while wq:
        emit_wdma(*wq.pop(0))

    # (u fp8 tensor, weight plane name) passes per accumulation group,
    # ordered by when the operands become available (q before residual)
    PASSES = (
        ("t1qq", u1q, "c1q"), ("t2qq", u2q, "c2q"), ("t3qq", u3q, "c3q"),
        ("t2qr", u2q, "c2r"), ("t3qr", u3q, "c3r"),
        ("t2rq", u2r, "c2q"), ("t3rq", u3r, "c3q"),
    )

    def emit_colsum(oh):
        # bias: colsum of ssq+ssr via DR matmuls with all-ones stationary;
        # bias_sb holds bias/WSCALE (the yo stt adds it after descale)
        bias_ps = psum_b.tile([P, OW], FP32, tag="po",
                              name=f"bias_ps{oh}")
        n_cs = 2 * KP
        ci = 0
        for src in ("ssq", "ssr"):
            for kp in range(KP):
                nc.tensor.matmul(
                    bias_ps[:], ones_dr[:],
                    sssb[(src, oh)][:, 2 * kp:2 * kp + 2, :],
                    start=(ci == 0), stop=(ci == n_cs - 1),
                    perf_mode=mybir.MatmulPerfMode.DoubleRow)
                ci += 1
        nc.scalar.activation(out=bias_sb[oh][:], in_=bias_ps[:],
                             func=mybir.ActivationFunctionType.Copy,
                             scale=1.0 / WSCALE)
        if cfg["splits"]:
            nc.scalar.activation(out=bias_sc[oh][:], in_=bias_ps[:],
                                 func=mybir.ActivationFunctionType.Copy)

    def mm(po, u, cname, oh, kp, j, start, stop=False):
        nc.tensor.matmul(
            po[:],
            u[:, 2 * kp:2 * kp + 2, j * P:(j + 1) * P],
            wsb[(cname, oh)][:, 2 * kp:2 * kp + 2, :],
            start=start, stop=stop,
            perf_mode=mybir.MatmulPerfMode.DoubleRow)

    close_n = [0]

    def emit_close(po, oh, j, split=1):
        # yo = po/WSCALE + bias, alternating between the two late-phase-idle
        # elementwise engines so closes never serialize.  split>1 chops the
        # close into parallel column strips (tail-latency reduction for the
        # last groups).
        yo = yopool.tile([P, OW], BF16, tag="yo")
        step = OW // split
        for s in range(split):
            c = slice(s * step, (s + 1) * step)
            eng = yengs[close_n[0] % len(yengs)]
            close_n[0] += 1
            eng.scalar_tensor_tensor(out=yo[:, c], in0=po[:, c],
                                     scalar=1.0 / WSCALE,
                                     in1=bias_sb[oh][:, c],
                                     op0=MULT, op1=ADD)
            nc.sync.dma_start(
                out=y_d[j * P:(j + 1) * P,
                        oh * OW + s * step:oh * OW + (s + 1) * step],
                in_=yo[:, c])

    yengs = {"vector": (nc.vector,)}[cfg["yo_engine"]]

    # phase A (u-prep-gated): first NA groups of oh 0, ki-pair-major so PE
    # consumption paces with u production.  phase B (free-running): the
    # rest, group-major so groups close staggered and yo/DMA overlap PE.
    NA = min(cfg["wave_a"], NJ)
    NC_ = min(cfg["companions"], NJ - NA)  # q-only companion groups
    pos_a = {j: psum_o.tile([P, OW], FP32, tag="po", name=f"poA{j}")
             for j in range(NA + NC_)}
    # availability-ordered (pass-group, kp) interleave: q-passes stream off
    # DVE (fast), r-passes off the residual engine (slower).  Companion
    # groups join only the q-columns (their r-columns run at phase-B start,
    # filling early PE gaps without extra PSUM pressure later).
    qs, rs = PASSES[:3], PASSES[3:]
    if cfg["sched"] == "auto":
        # sort single-pass columns by estimated operand availability
        # (pair-rate ~6.9us on DVE/Pool; offsets from per-pair queue order)
        OFFS = {"t1qq": 6.3, "t2qq": 2.9, "t3qq": 4.7, "t2qr": 2.9,
                "t3qr": 4.7, "t2rq": 6.3, "t3rq": 6.3}
        PLANE = {"t1qq": 2.0, "t2qq": 1.0, "t3qq": 3.0, "t2qr": 5.0,
                 "t3qr": 6.0, "t2rq": 1.0, "t3rq": 3.0}
        cols = []
        for pi, p in enumerate(PASSES):
            for kp in range(KP):
                est = max(4.0 + 6.9 * kp + OFFS[p[0]], 2.0 + PLANE[p[0]])
                cols.append((est, kp, pi, p))
        cols.sort(key=lambda c: (c[0], c[1]))
        sched = [((p,), kp) for _, kp, _, p in cols]
    else:
        sched = [(qs, int(c[1:])) if c[0] == "q" else (rs, int(c[1:]))
                 for c in cfg["sched"].split()]
        sched = [(grp, kp) for grp, kp in sched if kp < KP]
    def grp_is_q(g):
        return all(p[0].endswith("qq") or p[0].endswith("qr") for p in g) \
            and g is not rs
    n_q = sum(len(g) for g, _ in sched if grp_is_q(g))
    n_r = sum(len(g) for g, _ in sched if not grp_is_q(g))
    total = {j: (n_q + n_r if j < NA else n_q + len(rs) * KP)
             for j in range(NA + NC_)}
    done = {j: 0 for j in range(NA + NC_)}

    def mm_a(j, u, cname, kp):
        done[j] += 1
        mm(pos_a[j], u, cname, 0, kp, j, start=(done[j] == 1),
           stop=(done[j] == total[j]))

    # Split groups: output tiles whose kp0..KPH-1 contribution runs during
    # phase A in a rotating PSUM bank, parked to an SBUF partial (ACT copy,
    # bank freed) and merged at the final close.  Gives PE fill work while
    # the u streams pace phase A.
    all_groups = [(0, j) for j in range(NJ)]
    all_groups += [(1, j) for j in range(NJ)] if OH > 1 else []
    split_groups = ([g for g in all_groups[NA + NC_:]][:cfg["splits"]]
                    if KP >= 2 else [])
    KPH = max(1, KP // 2)
    partials = {}

    def emit_split_early(oh, j):
        po = psum_o.tile([P, OW], FP32, tag="po", name=f"poS{oh}_{j}")
        n = 0
        for kp in range(KPH):
            for pi, (_, u, cname) in enumerate(PASSES):
                n += 1
                mm(po, u, cname, oh, kp, j, start=(n == 1),
                   stop=(n == KPH * len(PASSES)))
        part = singles.tile([P, OW], BF16, name=f"part{oh}_{j}")
        nc.scalar.activation(out=part[:], in_=po[:],
                             func=mybir.ActivationFunctionType.Copy,
                             scale=1.0 / WSCALE)
        partials[(oh, j)] = part

    def emit_split_final(oh, j):
        po = psum_o.tile([P, OW], FP32, tag="po", name=f"poF{oh}_{j}")
        n = 0
        for kp in range(KPH, KP):
            for pi, (_, u, cname) in enumerate(PASSES):
                n += 1
                mm(po, u, cname, oh, kp, j, start=(n == 1))
        nc.tensor.matmul(po[:], e_row[:], bias_sc[oh][:], start=False,
                         stop=True)
        yo = yopool.tile([P, OW], BF16, tag="yo")
        nc.vector.scalar_tensor_tensor(
            out=yo[:], in0=po[:], scalar=1.0 / WSCALE,
            in1=partials[(oh, j)][:], op0=MULT, op1=ADD)
        nc.sync.dma_start(
            out=y_d[j * P:(j + 1) * P, oh * OW:(oh + 1) * OW], in_=yo[:])

    # tail-pipeline: the first non-A group's columns that don't depend on
    # the last-arriving residuals run right before phase A's final columns,
    # filling the stream-tail PE gap (uses the one spare PSUM buffer)
    b0 = all_groups[NA + NC_] if (cfg["tailpipe"] and KP >= 2
                                  and len(all_groups) > NA + NC_
                                  and not split_groups) else None
    tp_si = max(0, len(sched) - cfg["tailpipe"]) if b0 else None
    po_b0 = None
    nb0 = 0

    splits_iter = list(split_groups)
    for si, (grp, kp) in enumerate(sched):
        if b0 is not None and si == tp_si:
            po_b0 = psum_o.tile([P, OW], FP32, tag="po",
                                name=f"poTP{b0[0]}_{b0[1]}")
            for grp2, kp2 in sched[:tp_si]:
                for _, u2, cn2 in grp2:
                    nb0 += 1
                    mm(po_b0, u2, cn2, b0[0], kp2, b0[1], start=(nb0 == 1))
        is_q = grp_is_q(grp)
        for _, u, cname in grp:
            for j in range(NA + (NC_ if is_q else 0)):
                mm_a(j, u, cname, kp)
        if si == cfg["colsum_at"]:
            emit_colsum(0)
        if si >= cfg["split_from"] and splits_iter:
            oh, j = splits_iter.pop(0)
            emit_split_early(oh, j)
    if cfg["colsum_at"] >= len(sched):
        emit_colsum(0)
    while splits_iter:
        emit_split_early(*splits_iter.pop(0))
    # companions: finish their r-columns, then close everything
    for kp in range(KP):
        for _, u, cname in rs:
            for j in range(NA, NA + NC_):
                mm_a(j, u, cname, kp)
    if b0 is not None:
        n_total = sum(len(g) for g, _ in sched)
        for grp2, kp2 in sched[tp_si:]:
            for _, u2, cn2 in grp2:
                nb0 += 1
                mm(po_b0, u2, cn2, b0[0], kp2, b0[1], start=False,
                   stop=(nb0 == n_total))
    for j in range(NA + NC_):
        emit_close(pos_a[j], 0, j)
    if b0 is not None:
        emit_close(po_b0, b0[0], b0[1])

    if OH > 1:
        emit_colsum(1)
    for oh, j in split_groups:
        emit_split_final(oh, j)
    for gi, (oh, j) in enumerate(all_groups[NA + NC_:]):
        if (oh, j) in partials or (b0 is not None and (oh, j) == b0):
            continue
        po = psum_o.tile([P, OW], FP32, tag="po", name=f"poB{oh}_{j}")
        for kp in range(KP):
            for pi, (_, u, cname) in enumerate(PASSES):
                mm(po, u, cname, oh, kp, j, start=(kp == 0 and pi == 0),
                   stop=(kp == KP - 1 and pi == len(PASSES) - 1))
        emit_close(po, oh, j)


W_PLANE_NAMES = ("c1q", "c2q", "c2r", "c3q", "c3r", "ssq", "ssr")


def build_nc(b_loc=B_FULL // N_CORES, i_dim=I_DIM, o_dim=O_DIM,
             n_cores=N_CORES, cfg=None):
    nc = bacc.Bacc("TRN2", target_bir_lowering=False, debug=False,
                   num_devices=n_cores)
    KI = i_dim // P
    xt_d = nc.dram_tensor("xt", [P, KI, b_loc], BF16,
                          kind="ExternalInput").ap()
    wplanes_d = {
        name: nc.dram_tensor(name, [P, KI, o_dim], FP8,
                             kind="ExternalInput").ap()
        for name in W_PLANE_NAMES
    }
    y_d = nc.dram_tensor("y", [b_loc, o_dim], BF16, kind="ExternalOutput").ap()
    with tile.TileContext(nc) as tc:
        _bessel_body(tc, y_d, xt_d, wplanes_d, b_loc, i_dim, o_dim, cfg=cfg)
    nc.compile()
    return nc


def prep_inputs(x, w, n_cores=N_CORES):
    """Host-side data prep: shard/permute x, fold + cast weight planes."""
    x = np.asarray(x, dtype=np.float32)
    w = np.asarray(w, dtype=np.float32)
    b_full, i_dim = x.shape
    o_dim = w.shape[1]
    KI = i_dim // P
    b_loc = b_full // n_cores

    # x^T permuted to [p, ki, b] (i = ki*P + p), cast bf16 (halves DMA)
    xt = np.ascontiguousarray(
        x.T.reshape(KI, P, b_full).transpose(1, 0, 2)).astype(BF16_NP)

    w64 = w.astype(np.float64)
    W0, W1, W2, W3 = (w64[..., d] for d in range(4))
    planes64 = {
        "c1": W1 + 3 * W2 + 6 * W3,
        "c2": W2 + 5 * W3,
        "c3": W3,
        "ss": W0 + W1 + W2 + W3,
    }

    def perm(a):  # [I, O] -> [p, ki, O]
        return np.ascontiguousarray(
            a.reshape(KI, P, o_dim).transpose(1, 0, 2))

    def to_fp8(a):  # saturating e4m3 cast (TRN max normal 240)
        return np.clip(a, -240.0, 240.0).astype(FP8_NP)

    out = {"c1q": perm(to_fp8(WSCALE * planes64["c1"]))}
    for name in ("c2", "c3", "ss"):
        hi64 = WSCALE * planes64[name]
        q = to_fp8(hi64)
        r = to_fp8(hi64 - q.astype(np.float64))
        out[name + "q"] = perm(q)
        out[name + "r"] = perm(r)

    in_maps = []
    for c in range(n_cores):
        m = {"xt": np.ascontiguousarray(
            xt[:, :, c * b_loc:(c + 1) * b_loc])}
        m.update(out)
        in_maps.append(m)
    return in_maps


_NC_CACHE = {}


def _get_nc():
    if "full" not in _NC_CACHE:
        _NC_CACHE["full"] = build_nc()
    return _NC_CACHE["full"]


def run_spmd(x, bessel_coeffs, trace=False, **kwargs):
    from concourse.bass_utils import run_bass_kernel_spmd

    nc = _get_nc()
    in_maps = prep_inputs(x, bessel_coeffs)
    res = run_bass_kernel_spmd(nc, in_maps, core_ids=list(range(N_CORES)),
                               trace=trace, **kwargs)
    y = np.concatenate(
        [np.asarray(r["y"]).astype(np.float32) for r in res.results], axis=0)
    return y, res


def kernel(x, bessel_coeffs):
    y, _ = run_spmd(x, bessel_coeffs)
    return y.astype(np.float32)


def _ref_np(x, w):
    t = np.tanh(np.asarray(x, dtype=np.float64))
    w = np.asarray(w, dtype=np.float64)
    basis = [np.ones_like(t), t + 1.0]
    for i in range(2, NDEG):
        basis.append((2 * i - 1) * t * basis[i - 1] + basis[i - 2])
    bz = np.stack(basis, axis=-1)
    return np.einsum("bid,iod->bo", bz, w)


def _selftest_sim(b_loc=256, i_dim=256, o_dim=1024):
    """CoreSim check on a small config exercising all loop paths."""
    from concourse.bass_interp import CoreSim

    nc = build_nc(b_loc=b_loc, i_dim=i_dim, o_dim=o_dim, n_cores=1)
    rng = np.random.default_rng(0)
    x = rng.standard_normal((b_loc, i_dim)).astype(np.float32)
    w = (rng.standard_normal((i_dim, o_dim, NDEG)) / (i_dim * NDEG)).astype(
        np.float32)
    in_maps = prep_inputs(x, w, n_cores=1)
    sim = CoreSim(nc)
    for name, arr in in_maps[0].items():
        sim.tensor(name)[:] = arr
    sim.simulate()
    y = np.array(sim.tensor("y")).astype(np.float64)
    ref = _ref_np(x, w)
    scale = np.abs(ref).max()
    err = np.abs(y - ref).max() / scale
    print(f"sim scale={scale:.4g} max_abs_rel_err={err:.4g}")
    assert err < 2e-2, err
    print("SIM OK")


def _timesim(cfg=None):
    from concourse.timeline_sim import TimelineSim

    nc = build_nc(cfg=cfg)
    t = TimelineSim(nc).simulate()
    print(f"TimelineSim: {t:.0f} ns")
    return t


if __name__ == "__main__":
    if "--sim" in sys.argv:
        _selftest_sim()
    if "--timesim" in sys.argv:
        _timesim()
